# revision 13
# baseline (speedup 1.0000x reference)
"""Chamfer + rate-distortion loss kernel for Trainium2 (8 NeuronCores, SPMD).

Strategy (data-parallel over batch N=8, one point-cloud pair per core):
  - Negated squared distances G[p,q] = 2*x_p.y_q - |x_p|^2 - |y_q|^2 = -d[p,q]
    computed as ONE K=18 bf16 matmul per 128x512 tile (1 cycle/row on PE).
    Coordinates are split hi/lo into bf16 pairs (x ~ xh + xl to ~2^-16 rel),
    so the PE computes the EXACT squared distance between the slightly
    perturbed points x~, y~ (norm rows are derived from x~ itself, split into
    3 bf16 terms).  Near the minimum, the error scales with |x-y| -> no
    meaningful min-selection bias (plain fp32/f32r matmuls are resp. 4x too
    slow / TF32-rounded with ~1e-3 abs error, 14% chamfer error).
        lhsT rows: [xh(3), xh(3), xl(3), xl(3), x2h, x2m, x2l, 1, 1, 1]
        rhs  rows: [2yh(3), 2yl(3), 2yh(3), 2yl(3), -1,-1,-1, -y2{h,m,l}]
  - ACT copies each 2-bank PSUM group to fp16 SBUF staging; the DVE runs two
    fast fp16 ops per group:
        rowslots[:, i, jg] = max_q G  (free-axis reduce -> cham_x)
        colaccW[jg] = max(G, colaccW) (elementwise max over i -> cham_y)
  - Column direction finalized with PE transposes + free-axis max.
  - Norm rows are computed on device in a [128,32,3] layout and flattened into
    the [1,4096] matmul operand rows by SBUF->SBUF DMAs (partition-major
    flatten == natural point order).
  - log-likelihood term: ACT Ln with accum_out (free-axis sum).
  - Final cross-partition sums via gpsimd partition_all_reduce; host combines
    the 8 per-core partial sums into the 5 scalar outputs.
"""

import math

import numpy as np

N_CORES = 8
P = 4096  # points per cloud
C = 3
M = 2048  # likelihood entries per batch row
PT = 128  # p-tile (matmul stationary free dim -> PSUM partitions)
QT = 512  # q-tile (matmul moving free dim -> one PSUM bank)
NI = P // PT  # 32
NJ = P // QT  # 8
TW = P // 128  # 32: free width of the [128, TW, C] coord layout
LIK_W = M // 128  # 16
BIG_NEG = -3.0e38
LMBDA = 1.0

_CACHE = {}


def _build_nc():
    import concourse.mybir as mybir
    from concourse import bacc, bass_isa
    from concourse.masks import make_identity
    from concourse.tile import TileContext

    f32 = mybir.dt.float32
    bf16 = mybir.dt.bfloat16
    AX = mybir.AxisListType
    OP = mybir.AluOpType
    ACT = mybir.ActivationFunctionType

    nc = bacc.Bacc(trn_type="TRN2")

    xa_d = nc.dram_tensor("xa", [18, P], bf16, kind="ExternalInput")
    ya_d = nc.dram_tensor("ya", [18, P], bf16, kind="ExternalInput")
    xpm_d = nc.dram_tensor("xpm", [128, 2, TW, C], bf16, kind="ExternalInput")
    ypm_d = nc.dram_tensor("ypm", [128, 2, TW, C], bf16, kind="ExternalInput")
    lik_d = nc.dram_tensor("lik", [128, LIK_W], f32, kind="ExternalInput")
    out_d = nc.dram_tensor("out", [1, 3], f32, kind="ExternalOutput")

    with TileContext(nc) as tc:
        with (
            tc.tile_pool(name="ops", bufs=1) as ops,
            tc.tile_pool(name="work", bufs=1) as work,
            tc.tile_pool(name="acc", bufs=1) as accp,
            tc.tile_pool(name="cap", bufs=2) as cap,
            tc.tile_pool(name="psum", bufs=3, space="PSUM") as psum,
            tc.tile_pool(name="psumt", bufs=2, space="PSUM") as psumt,
        ):
            # ---------- likelihood term (independent; emitted first) ----------
            lik = ops.tile([128, LIK_W], f32)
            nc.sync.dma_start(out=lik, in_=lik_d[:, :])
            ln_scratch = work.tile([128, LIK_W], f32)
            likacc = accp.tile([128, 1], f32)
            nc.scalar.activation(
                out=ln_scratch, in_=lik, func=ACT.Ln, accum_out=likacc
            )

            # ---------- operand load + norm rows ----------
            xa = ops.tile([18, P], bf16)
            ya = ops.tile([18, P], bf16)
            nc.sync.dma_start(out=xa[0:12, :], in_=xa_d[0:12, :])
            nc.sync.dma_start(out=xa[15:18, :], in_=xa_d[15:18, :])
            nc.sync.dma_start(out=ya[0:15, :], in_=ya_d[0:15, :])

            # |x~|^2 from the reconstructed bf16-split coords, split into
            # 3 bf16 terms, flattened partition-major into the operand rows
            # ([128,32] -> [1,4096] is natural point order).
            for side, pm_d, dst, sgn in (
                ("x", xpm_d, xa, 1.0),
                ("y", ypm_d, ya, -1.0),
            ):
                pm = work.tile([128, 2, TW, C], bf16, name=f"{side}pm_t")
                nc.sync.dma_start(out=pm, in_=pm_d[:, :, :, :])
                rec = work.tile([128, TW, C], f32, name=f"{side}rec")
                nc.vector.tensor_tensor(
                    out=rec, in0=pm[:, 0], in1=pm[:, 1], op=OP.add
                )
                sq = work.tile([128, TW, C], f32, name=f"{side}sq")
                nc.vector.tensor_mul(out=sq, in0=rec, in1=rec)
                n2 = work.tile([128, TW], f32, name=f"{side}n2")
                nc.vector.tensor_reduce(out=n2, in_=sq, axis=AX.X, op=OP.add)
                if sgn < 0:
                    nc.scalar.mul(out=n2, in_=n2, mul=-1.0)
                row0 = 12 if sgn > 0 else 15
                res = n2
                for t in range(3):
                    term = work.tile([128, TW], bf16, name=f"{side}t{t}")
                    nc.vector.tensor_copy(out=term, in_=res)
                    nc.sync.dma_start(
                        out=dst[row0 + t : row0 + t + 1, :], in_=term
                    )
                    if t < 2:
                        nres = work.tile([128, TW], f32, name=f"{side}r{t}")
                        nc.vector.tensor_tensor(
                            out=nres, in0=res, in1=term, op=OP.subtract
                        )
                        res = nres

            identity = ops.tile([128, 128], f32)
            make_identity(nc, identity)

            # ---------- main loop ----------
            # q-chunks are grouped 2 at a time (GW = 1024 = 2 PSUM banks).
            # ACT copies each PSUM group to fp16 SBUF staging; the DVE then
            # runs at 2x/4x fp16 speed:
            #   rowslots[:, i, jg] = max over the group's 1024 q of G
            #   colaccW[jg]        = elementwise max over i   (fp16)
            GW = 2 * QT  # 1024
            NG = P // GW  # 4 groups
            NB = GW // 128  # 8 transpose blocks per group
            f16 = mybir.dt.float16
            rowslots = accp.tile([128, NI, NG], f32)
            colq = accp.tile([128, NG * NB], f32)

            for jg in range(NG):
                caw = cap.tile([128, GW], f16, tag="caw")
                for i in range(NI):
                    psq = psum.tile([128, GW], f32, tag="ps", bufs=3)
                    for jj in range(GW // QT):
                        nc.tensor.matmul(
                            psq[:, jj * QT : (jj + 1) * QT],
                            xa[:, i * PT : (i + 1) * PT],
                            ya[
                                :,
                                jg * GW + jj * QT : jg * GW + (jj + 1) * QT,
                            ],
                        )
                    stg = cap.tile([128, GW], f16, tag="stg", bufs=3)
                    nc.scalar.copy(out=stg, in_=psq)
                    nc.vector.tensor_reduce(
                        out=rowslots[:, i, jg : jg + 1],
                        in_=stg,
                        axis=AX.X,
                        op=OP.max,
                    )
                    if i == 0:
                        nc.vector.tensor_copy(out=caw, in_=stg)
                    else:
                        nc.vector.tensor_tensor(
                            out=caw, in0=stg, in1=caw, op=OP.max
                        )
                # finalize column direction for this group: convert to f32,
                # transpose 128x128 blocks on the PE, then free-axis max
                cf = cap.tile([128, NB, 128], f32, tag="cf")
                nc.scalar.copy(out=cf, in_=caw)
                for tb in range(NB // 4):
                    tp = psumt.tile([128, 4, 128], f32, tag="tp")
                    for b in range(4):
                        nc.tensor.transpose(
                            tp[:, b, :], cf[:, tb * 4 + b, :], identity
                        )
                    nc.vector.tensor_reduce(
                        out=colq[
                            :, jg * NB + tb * 4 : jg * NB + (tb + 1) * 4
                        ],
                        in_=tp,
                        axis=AX.X,
                        op=OP.max,
                    )

            # ---------- final scalars ----------
            rowacc = accp.tile([128, NI], f32)
            nc.vector.tensor_reduce(out=rowacc, in_=rowslots, axis=AX.X, op=OP.max)
            finals = accp.tile([128, 3], f32)
            nc.vector.tensor_reduce(
                out=finals[:, 0:1], in_=rowacc, axis=AX.X, op=OP.add
            )
            nc.vector.tensor_reduce(
                out=finals[:, 1:2], in_=colq, axis=AX.X, op=OP.add
            )
            nc.vector.tensor_copy(out=finals[:, 2:3], in_=likacc)
            finals2 = accp.tile([128, 3], f32)
            nc.gpsimd.partition_all_reduce(
                out_ap=finals2,
                in_ap=finals,
                channels=128,
                reduce_op=bass_isa.ReduceOp.add,
            )
            nc.sync.dma_start(out=out_d[:, :], in_=finals2[0:1, :])

    nc.compile()
    return nc


def _split_bf16(v):
    """v (f32) ~ hi + lo with both terms bf16; returns f32 arrays."""
    import ml_dtypes

    bf = ml_dtypes.bfloat16
    hi = v.astype(bf).astype(np.float32)
    lo = (v - hi).astype(bf).astype(np.float32)
    return hi, lo


def _prepare_in_maps(x_hat, points, lik_y):
    import ml_dtypes

    bf = ml_dtypes.bfloat16
    in_maps = []
    for n in range(N_CORES):
        x = np.ascontiguousarray(x_hat[n], dtype=np.float32)  # [P, 3]
        y = np.ascontiguousarray(points[n], dtype=np.float32)
        xh, xl = _split_bf16(x)
        yh, yl = _split_bf16(y)
        xa = np.zeros((18, P), dtype=np.float32)
        xa[0:3] = xh.T
        xa[3:6] = xh.T
        xa[6:9] = xl.T
        xa[9:12] = xl.T
        # rows 12-14: x2 h/m/l, filled on device
        xa[15:18] = 1.0
        ya = np.zeros((18, P), dtype=np.float32)
        ya[0:3] = 2.0 * yh.T
        ya[3:6] = 2.0 * yl.T
        ya[6:9] = 2.0 * yh.T
        ya[9:12] = 2.0 * yl.T
        ya[12:15] = -1.0
        # rows 15-17: -y2 h/m/l, filled on device
        xpm = np.stack(
            [xh.reshape(128, TW, C), xl.reshape(128, TW, C)], axis=1
        )
        ypm = np.stack(
            [yh.reshape(128, TW, C), yl.reshape(128, TW, C)], axis=1
        )
        in_maps.append(
            {
                "xa": xa.astype(bf),
                "ya": ya.astype(bf),
                "xpm": xpm.astype(bf),
                "ypm": ypm.astype(bf),
                "lik": np.ascontiguousarray(
                    lik_y[n], dtype=np.float32
                ).reshape(128, LIK_W),
            }
        )
    return in_maps


def _combine(outs):
    """outs: [8, 3] per-core (sum_p max_q G, sum_q max_p G, sum ln lik)."""
    outs = np.asarray(outs, dtype=np.float64)
    s_row = outs[:, 0].sum()
    s_col = outs[:, 1].sum()
    s_lik = outs[:, 2].sum()
    cham_x = -s_row / (N_CORES * P)
    cham_y = -s_col / (N_CORES * P)
    rec_loss = cham_x + cham_y
    bit_loss = s_lik / math.log(2.0) / (-N_CORES)
    bpp = bit_loss / P
    loss = bpp + LMBDA * rec_loss
    return (
        np.float32(loss),
        np.float32(bpp),
        np.float32(rec_loss),
        np.float32(bit_loss),
        np.float32(bpp),
    )


def kernel(x_hat, points, lik_y, _trace=False):
    from concourse.bass_utils import run_bass_kernel_spmd

    if "nc" not in _CACHE:
        _CACHE["nc"] = _build_nc()
    nc = _CACHE["nc"]
    in_maps = _prepare_in_maps(x_hat, points, lik_y)
    res = run_bass_kernel_spmd(
        nc, in_maps, core_ids=list(range(N_CORES)), trace=_trace
    )
    outs = np.stack([r["out"][0] for r in res.results])
    result = _combine(outs)
    if _trace:
        return result, res
    return result
